# revision 1
# baseline (speedup 1.0000x reference)
"""Trainium2 Bass kernel for nn_ASC_LSTM (per-step LSTM encoder/decoder).

Strategy: data-parallel over batch (32 rows/core x 8 cores), weights
replicated and streamed from HBM in bf16 (host-cast). Gates are computed
transposed ([gate_rows, batch] in PSUM) so the recurrent hidden state
stays in [128, 4, 32] K-chunk layout and never needs an on-chip
transpose. Per-step biases are applied by the ScalarE activation that
reads PSUM (sigmoid/tanh with per-partition bias operand).
"""
import os
import sys

import numpy as np
import ml_dtypes

sys.path.insert(0, "/opt/trn_rl_repo")

import concourse.bass as bass
import concourse.tile as tile
from concourse import bacc, mybir
from concourse import bass_utils
from concourse.bass import ts

B, I, H, S, RES = 256, 256, 512, 64, 4
NCORES = 8
BLOC = B // NCORES  # 32
BF16 = mybir.dt.bfloat16
F32 = mybir.dt.float32
AF = mybir.ActivationFunctionType

_STATE = {}


def _build_module():
    nc = bacc.Bacc(
        "TRN2",
        target_bir_lowering=False,
        debug=False,
        enable_asserts=False,
        num_devices=NCORES,
    )
    wt_d = nc.dram_tensor("wt", [S, 128, 6, 16 * 128], BF16, kind="ExternalInput").ap()
    wdt_d = nc.dram_tensor("wdt", [S, 128, 4, 6 * 128], BF16, kind="ExternalInput").ap()
    x_d = nc.dram_tensor("xr", [128, S, 2, BLOC], BF16, kind="ExternalInput").ap()
    benc_d = nc.dram_tensor("benc", [128, S, 16], F32, kind="ExternalInput").ap()
    bdec_d = nc.dram_tensor("bdec", [128, S, 6], F32, kind="ExternalInput").ap()
    out_d = nc.dram_tensor("out", [S, 128, 2, BLOC], F32, kind="ExternalOutput").ap()

    with tile.TileContext(nc) as tc:
        with (
            tc.tile_pool(name="wenc", bufs=4) as wpool,
            tc.tile_pool(name="wdec", bufs=4) as wdpool,
            tc.tile_pool(name="big", bufs=1) as bigpool,
            tc.tile_pool(name="gates", bufs=3) as gpool,
            tc.tile_pool(name="small", bufs=3) as spool,
            tc.tile_pool(name="psum", bufs=8, space="PSUM") as psum,
        ):
            x_sb = bigpool.tile([128, S, 2, BLOC], BF16, tag="xsb")
            nc.sync.dma_start(out=x_sb, in_=x_d)
            benc_sb = bigpool.tile([128, S, 16], F32, tag="benc")
            nc.sync.dma_start(out=benc_sb, in_=benc_d)
            bdec_sb = bigpool.tile([128, S, 6], F32, tag="bdec")
            nc.sync.dma_start(out=bdec_sb, in_=bdec_d)

            enc_hist = bigpool.tile([128, S, 4, BLOC], F32, tag="ehist")
            enc_bf = bigpool.tile([128, S, 4, BLOC], BF16, tag="ebf")

            # ---------------- encoder scan ----------------
            h_bf_prev = None
            for t in range(S):
                w_sb = wpool.tile([128, 6, 16 * 128], BF16, tag="wenc")
                nks = 2 if t == 0 else 6
                # k=0..3 (x + first h chunks): 8 fine-grained HWDGE transfers;
                # k=4,5: 2 large SWDGE transfers (GpSimd issue cost ~1us each)
                for k in range(min(nks, 4)):
                    for hh in range(2):
                        nc.sync.dma_start(
                            out=w_sb[:, k, ts(hh, 8 * 128)],
                            in_=wt_d[t, :, k, ts(hh, 8 * 128)],
                        )
                for k in range(4, nks):
                    nc.gpsimd.dma_start(out=w_sb[:, k], in_=wt_d[t, :, k])
                sig_i = gpool.tile([128, 4, BLOC], F32, tag="sig_i")
                sig_f = gpool.tile([128, 4, BLOC], F32, tag="sig_f")
                tanh_g = gpool.tile([128, 4, BLOC], F32, tag="tanh_g")
                sig_o = gpool.tile([128, 4, BLOC], F32, tag="sig_o")
                for m in range(16):
                    ps = psum.tile([128, BLOC], F32, tag="ps")
                    for k in range(nks):
                        rhs = x_sb[:, t, k] if k < 2 else h_bf_prev[:, k - 2]
                        nc.tensor.matmul(
                            ps,
                            lhsT=w_sb[:, k, ts(m, 128)],
                            rhs=rhs,
                            start=(k == 0),
                            stop=(k == nks - 1),
                        )
                    # bias-add on DVE (per-partition scalar); activations are
                    # applied batched per gate tile below on ScalarE
                    dst = (
                        sig_i[:, m]
                        if m < 4
                        else sig_f[:, m - 4]
                        if m < 8
                        else tanh_g[:, m - 8]
                        if m < 12
                        else sig_o[:, m - 12]
                    )
                    nc.vector.tensor_scalar_add(dst, ps, benc_sb[:, t, m : m + 1])
                nc.scalar.activation(out=sig_i, in_=sig_i, func=AF.Sigmoid)
                nc.scalar.activation(out=sig_f, in_=sig_f, func=AF.Sigmoid)
                nc.scalar.activation(out=tanh_g, in_=tanh_g, func=AF.Tanh)
                nc.scalar.activation(out=sig_o, in_=sig_o, func=AF.Sigmoid)
                # c = sig_f * h_prev + sig_i * tanh_g ; h = sig_o * tanh(c)
                nc.vector.tensor_mul(sig_i, sig_i, tanh_g)
                if t > 0:
                    nc.vector.tensor_mul(sig_f, sig_f, enc_hist[:, t - 1])
                    nc.vector.tensor_add(sig_i, sig_i, sig_f)
                nc.scalar.activation(out=sig_i, in_=sig_i, func=AF.Tanh)
                nc.vector.tensor_mul(enc_hist[:, t], sig_i, sig_o)
                h_bf_prev = gpool.tile([128, 4, BLOC], BF16, tag="hbf")
                nc.vector.tensor_copy(out=h_bf_prev, in_=enc_hist[:, t])
                # fused elu -> bf16 history for the decoder (keeps the raw h
                # in enc_hist for the next-step recurrence)
                etmp = spool.tile([128, 4, BLOC], F32, tag="elut")
                hpos = spool.tile([128, 4, BLOC], F32, tag="hpos")
                nc.vector.tensor_scalar_min(etmp, enc_hist[:, t], 0.0)
                nc.scalar.activation(out=etmp, in_=etmp, func=AF.Exp)
                nc.vector.tensor_scalar_max(hpos, enc_hist[:, t], 0.0)
                nc.vector.tensor_add(hpos, hpos, etmp)
                nc.vector.tensor_scalar_add(enc_bf[:, t], hpos, -1.0)

            # ---------------- skip blend (bf16) ----------------
            for k in range(0, S, RES):
                nc.vector.tensor_add(
                    enc_bf[:, k], enc_bf[:, k], enc_bf[:, (k - RES) % S]
                )
                nc.vector.tensor_scalar_mul(enc_bf[:, k], enc_bf[:, k], 0.5)

            # ---------------- decoder (parallel over idx) ----------------
            run = None
            for idx in range(S):
                tsrc = S - 1 - idx
                wd_sb = wdpool.tile([128, 4, 6 * 128], BF16, tag="wdec")
                for k in range(3):
                    for hh in range(2):
                        nc.sync.dma_start(
                            out=wd_sb[:, k, ts(hh, 3 * 128)],
                            in_=wdt_d[idx, :, k, ts(hh, 3 * 128)],
                        )
                nc.gpsimd.dma_start(out=wd_sb[:, 3], in_=wdt_d[idx, :, 3])
                sid = spool.tile([128, 2, BLOC], F32, tag="sid")
                tgd = spool.tile([128, 2, BLOC], F32, tag="tgd")
                sod = spool.tile([128, 2, BLOC], F32, tag="sod")
                for m in range(6):
                    ps = psum.tile([128, BLOC], F32, tag="ps")
                    for k in range(4):
                        nc.tensor.matmul(
                            ps,
                            lhsT=wd_sb[:, k, ts(m, 128)],
                            rhs=enc_bf[:, tsrc, k],
                            start=(k == 0),
                            stop=(k == 3),
                        )
                    dst = (
                        sid[:, m]
                        if m < 2
                        else tgd[:, m - 2]
                        if m < 4
                        else sod[:, m - 4]
                    )
                    nc.vector.tensor_scalar_add(dst, ps, bdec_sb[:, idx, m : m + 1])
                nc.scalar.activation(out=sid, in_=sid, func=AF.Sigmoid)
                nc.scalar.activation(out=tgd, in_=tgd, func=AF.Tanh)
                nc.scalar.activation(out=sod, in_=sod, func=AF.Sigmoid)
                nc.vector.tensor_mul(sid, sid, tgd)  # c
                nc.scalar.activation(out=sid, in_=sid, func=AF.Tanh)
                if idx % RES == 0:
                    run = spool.tile([128, 2, BLOC], F32, tag="run")
                    nc.vector.tensor_mul(run, sid, sod)
                else:
                    nc.vector.tensor_mul(sid, sid, sod)  # hd
                    nc.vector.tensor_add(run, run, sid)
                outv = spool.tile([128, 2, BLOC], F32, tag="outv")
                nc.scalar.activation(out=outv, in_=run, func=AF.Tanh)
                nc.sync.dma_start(out=out_d[idx], in_=outv)
    nc.finalize()
    return nc


def _host_prep(inputs):
    bf = ml_dtypes.bfloat16
    W_all = np.concatenate([inputs["Wih_enc"], inputs["Whh_enc"]], axis=2)
    wt = np.ascontiguousarray(
        W_all.transpose(0, 2, 1).reshape(S, 6, 128, 16 * 128).transpose(0, 2, 1, 3)
    ).astype(bf)
    benc = np.ascontiguousarray(
        (inputs["bih_enc"] + inputs["bhh_enc"]).reshape(S, 16, 128).transpose(2, 0, 1)
    ).astype(np.float32)
    Wd = inputs["Wih_dec"]
    Wd2 = np.concatenate([Wd[:, 0:256], Wd[:, 512:1024]], axis=1)
    wdt = np.ascontiguousarray(
        Wd2.transpose(0, 2, 1).reshape(S, 4, 128, 6 * 128).transpose(0, 2, 1, 3)
    ).astype(bf)
    bd = inputs["bih_dec"] + inputs["bhh_dec"]
    bdec = np.ascontiguousarray(
        np.concatenate([bd[:, 0:256], bd[:, 512:1024]], axis=1)
        .reshape(S, 6, 128)
        .transpose(2, 0, 1)
    ).astype(np.float32)
    xr = np.ascontiguousarray(
        inputs["x"].reshape(B, 2, 128, S).transpose(2, 3, 1, 0)
    ).astype(bf)
    return wt, benc, wdt, bdec, xr


def kernel(**inputs):
    inputs = {k: np.asarray(v) for k, v in inputs.items()}
    if "nc" not in _STATE:
        _STATE["nc"] = _build_module()
    nc = _STATE["nc"]
    wt, benc, wdt, bdec, xr = _host_prep(inputs)
    in_maps = []
    for c in range(NCORES):
        in_maps.append(
            {
                "wt": wt,
                "wdt": wdt,
                "benc": benc,
                "bdec": bdec,
                "xr": np.ascontiguousarray(xr[:, :, :, c * BLOC : (c + 1) * BLOC]),
            }
        )
    res = bass_utils.run_bass_kernel_spmd(
        nc,
        in_maps,
        core_ids=list(range(NCORES)),
        trace=bool(int(os.environ.get("BASS_KERNEL_TRACE", "0"))),
    )
    _STATE["last_results"] = res
    outs = []
    for c in range(NCORES):
        o = res.results[c]["out"]  # [S,128,2,BLOC]
        outs.append(
            np.ascontiguousarray(
                o.transpose(3, 2, 1, 0).reshape(BLOC, 2 * 128, S)[:, :, ::-1]
            )
        )
    return np.concatenate(outs, axis=0).astype(np.float32)



# revision 4
# speedup vs baseline: 2.0442x; 2.0442x over previous
"""Trainium2 Bass kernel for nn_ASC_LSTM (per-step LSTM encoder/decoder).

Strategy: data-parallel over batch (32 rows/core x 8 cores). Weights are
replicated, host-scaled by 64 and quantized to fp8 e3m4 (all 16-bit on-chip surfaces use float16,
whose 10-bit mantissa keeps the recurrent-chain rounding error ~8x
below bfloat16), then streamed
from HBM in 4-step chunks (one large DMA per chunk, double-buffered).
Gates are computed transposed ([gate_rows, batch] in PSUM) with all 16
gate chunks of a step accumulated into a single PSUM bank; the per-step
bias is folded in with one K=16 matmul against a one-hot "ones" tensor.
Gate order is [i, f, o, g] so one sigmoid covers chunks 0:12 and one
tanh covers 12:16 (PSUM read + 1/64 descale fused into the activation).
The elu is batched per 16 steps (its Exp needs a different activation
table than sigmoid/tanh); its "-1" is folded into the decoder bias on
the host via row sums of the quantized decoder weights, which is exact
because the skip blend coefficients sum to 1.
"""
import os
import sys

import numpy as np
import ml_dtypes

sys.path.insert(0, "/opt/trn_rl_repo")

import concourse.bass as bass
import concourse.tile as tile
from concourse import bacc, mybir
from concourse import bass_utils

B, I, H, S, RES = 256, 256, 512, 64, 4
NCORES = 8
BLOC = B // NCORES  # 32
CH = 4   # encoder steps per weight-DMA chunk
DCH = 4  # decoder idxs per weight-DMA chunk
WSCALE = 64.0
F16 = mybir.dt.float16
F32 = mybir.dt.float32
FP8 = mybir.dt.float8e3
AF = mybir.ActivationFunctionType

_STATE = {}


def _build_module():
    nc = bacc.Bacc(
        "TRN2",
        target_bir_lowering=False,
        debug=False,
        enable_asserts=False,
        num_devices=NCORES,
    )
    wt_d = nc.dram_tensor("wt", [S // CH, 128, CH, 6, 16, 128], FP8, kind="ExternalInput").ap()
    wdt_d = nc.dram_tensor("wdt", [S // DCH, 128, DCH, 4, 6, 128], FP8, kind="ExternalInput").ap()
    x_d = nc.dram_tensor("xr", [128, S, 2, BLOC], F16, kind="ExternalInput").ap()
    benc_d = nc.dram_tensor("benc", [16, S, 128], F16, kind="ExternalInput").ap()
    bdec_d = nc.dram_tensor("bdec", [S // DCH, 6, DCH, 128], F32, kind="ExternalInput").ap()
    eones_d = nc.dram_tensor("eones", [16, 16, BLOC], F16, kind="ExternalInput").ap()
    dones_d = nc.dram_tensor("dones", [6, 6, BLOC], F32, kind="ExternalInput").ap()
    out_d = nc.dram_tensor("out", [128, S, 2, BLOC], F32, kind="ExternalOutput").ap()

    inv = 1.0 / WSCALE

    with tile.TileContext(nc) as tc:
        with (
            tc.tile_pool(name="wenc", bufs=2) as wpool,
            tc.tile_pool(name="wdec", bufs=2) as wdpool,
            tc.tile_pool(name="bdec", bufs=2) as bdpool,
            tc.tile_pool(name="big", bufs=1) as bigpool,
            tc.tile_pool(name="gates", bufs=2) as gpool,
            tc.tile_pool(name="small", bufs=2) as spool,
            tc.tile_pool(name="psum", bufs=4, space="PSUM") as psum,
        ):
            x_sb = bigpool.tile([128, S, 2, BLOC], F16, tag="xsb")
            nc.sync.dma_start(out=x_sb, in_=x_d)
            benc_sb = bigpool.tile([16, S, 128], F16, tag="benc")
            nc.sync.dma_start(out=benc_sb, in_=benc_d)
            eones_sb = bigpool.tile([16, 16, BLOC], F16, tag="eones")
            nc.sync.dma_start(out=eones_sb, in_=eones_d)
            dones_sb = bigpool.tile([6, 6, BLOC], F32, tag="dones")
            nc.sync.dma_start(out=dones_sb, in_=dones_d)

            # bf16 h history; becomes u = elu(h)+1 in place, then blended.
            hist = bigpool.tile([128, S, 4, BLOC], F16, tag="hist")
            out_sb = bigpool.tile([128, S, 2, BLOC], F32, tag="outsb")

            # ---------------- encoder scan ----------------
            h_f32 = None
            hb_prev = None
            for c in range(S // CH):
                w_sb = wpool.tile([128, CH, 6, 16, 128], FP8, tag="w")
                nc.sync.dma_start(out=w_sb, in_=wt_d[c])
                for i in range(CH):
                    t = c * CH + i
                    nks = 2 if t == 0 else 6
                    ps = psum.tile([128, 16, BLOC], F32, tag="ps")
                    nc.tensor.matmul(
                        ps, lhsT=benc_sb[:, t], rhs=eones_sb,
                        start=True, stop=False, skip_group_check=True,
                    )
                    for m in range(16):
                        for k in range(nks):
                            rhs = x_sb[:, t, k] if k < 2 else hb_prev[:, k - 2]
                            nc.tensor.matmul(
                                ps[:, m], lhsT=w_sb[:, i, k, m], rhs=rhs,
                                start=False, stop=(k == nks - 1),
                                skip_group_check=True,
                            )
                    gs = gpool.tile([128, 16, BLOC], F32, tag="gs")
                    nc.scalar.activation(out=gs[:, 0:12], in_=ps[:, 0:12], func=AF.Sigmoid, scale=inv)
                    nc.scalar.activation(out=gs[:, 12:16], in_=ps[:, 12:16], func=AF.Tanh, scale=inv)
                    # c = f*h_prev + i*g ; h = o*tanh(c)
                    cc = spool.tile([128, 4, BLOC], F32, tag="cc")
                    nc.vector.tensor_mul(cc, gs[:, 0:4], gs[:, 12:16])
                    if t > 0:
                        fh = spool.tile([128, 4, BLOC], F32, tag="fh")
                        nc.vector.tensor_mul(fh, gs[:, 4:8], h_f32)
                        nc.vector.tensor_add(cc, cc, fh)
                    tct = spool.tile([128, 4, BLOC], F32, tag="tct")
                    nc.scalar.activation(out=tct, in_=cc, func=AF.Tanh)
                    h_f32 = spool.tile([128, 4, BLOC], F32, tag="h32")
                    nc.vector.tensor_mul(h_f32, tct, gs[:, 8:12])
                    # bf16 copies: rolling tile for next-step matmul rhs, and
                    # the history slot (overwritten by elu/blend later)
                    hb_prev = spool.tile([128, 4, BLOC], F16, tag="hb")
                    nc.vector.tensor_copy(out=hb_prev, in_=h_f32)
                    nc.vector.tensor_copy(out=hist[:, t], in_=h_f32)
                    # batched elu for the finished 16-step block:
                    # u = relu(h) + exp(min(h,0))   (the -1 is folded into
                    # the decoder bias on host)
                    if t % 16 == 15:
                        blk = hist[:, t - 15 : t + 1]
                        en = spool.tile([128, 16, 4, BLOC], F16, tag="eneg")
                        nc.vector.tensor_scalar_min(en, blk, 0.0)
                        nc.scalar.activation(out=en, in_=en, func=AF.Exp)
                        nc.vector.tensor_scalar_max(blk, blk, 0.0)
                        nc.vector.tensor_add(blk, blk, en)

            # ---------------- skip blend (sequential chain) ----------------
            for k in range(0, S, RES):
                nc.vector.tensor_add(hist[:, k], hist[:, k], hist[:, (k - RES) % S])
                nc.vector.tensor_scalar_mul(hist[:, k], hist[:, k], 0.5)

            # ---------------- decoder (parallel over idx) ----------------
            run = None
            for c in range(S // DCH):
                wd_sb = wdpool.tile([128, DCH, 4, 6, 128], FP8, tag="wd")
                nc.sync.dma_start(out=wd_sb, in_=wdt_d[c])
                bd_sb = bdpool.tile([6, DCH, 128], F32, tag="bd")
                nc.sync.dma_start(out=bd_sb, in_=bdec_d[c])
                for j in range(DCH):
                    idx = c * DCH + j
                    tsrc = S - 1 - idx
                    psd = psum.tile([128, 6, BLOC], F32, tag="psd")
                    nc.tensor.matmul(
                        psd, lhsT=bd_sb[:, j], rhs=dones_sb,
                        start=True, stop=False, skip_group_check=True,
                    )
                    for m in range(6):
                        for k in range(4):
                            nc.tensor.matmul(
                                psd[:, m], lhsT=wd_sb[:, j, k, m], rhs=hist[:, tsrc, k],
                                start=False, stop=(k == 3), skip_group_check=True,
                            )
                    gd = gpool.tile([128, 6, BLOC], F32, tag="gd")
                    nc.scalar.activation(out=gd[:, 0:4], in_=psd[:, 0:4], func=AF.Sigmoid, scale=inv)
                    nc.scalar.activation(out=gd[:, 4:6], in_=psd[:, 4:6], func=AF.Tanh, scale=inv)
                    cd = spool.tile([128, 2, BLOC], F32, tag="cd")
                    nc.vector.tensor_mul(cd, gd[:, 0:2], gd[:, 4:6])
                    nc.scalar.activation(out=cd, in_=cd, func=AF.Tanh)
                    if idx % RES == 0:
                        run = spool.tile([128, 2, BLOC], F32, tag="run")
                        nc.vector.tensor_mul(run, cd, gd[:, 2:4])
                    else:
                        nc.vector.tensor_mul(cd, cd, gd[:, 2:4])
                        nc.vector.tensor_add(run, run, cd)
                    nc.scalar.activation(out=out_sb[:, idx], in_=run, func=AF.Tanh)
            nc.sync.dma_start(out=out_d, in_=out_sb)
    nc.finalize()
    return nc


def _host_prep(inputs):
    f16 = np.float16
    f8 = ml_dtypes.float8_e3m4
    # encoder: gate order [i, f, o, g]
    eperm = np.r_[0:512, 512:1024, 1536:2048, 1024:1536]
    W_all = np.concatenate([inputs["Wih_enc"], inputs["Whh_enc"]], axis=2)[:, eperm, :] * WSCALE
    # [t, 16m, 128q, 6k, 128p] -> [t, p, k, m, q]
    wt = np.ascontiguousarray(
        W_all.reshape(S, 16, 128, 6, 128).transpose(0, 4, 3, 1, 2)
    ).astype(f8)
    wt = np.ascontiguousarray(
        wt.reshape(S // CH, CH, 128, 6, 16, 128).transpose(0, 2, 1, 3, 4, 5)
    )
    benc = np.ascontiguousarray(
        ((inputs["bih_enc"] + inputs["bhh_enc"])[:, eperm] * WSCALE)
        .reshape(S, 16, 128)
        .transpose(1, 0, 2)
    ).astype(f16)
    eones = np.ascontiguousarray(
        np.repeat(np.eye(16, dtype=np.float32)[:, :, None], BLOC, axis=2)
    ).astype(f16)
    # decoder: gate order [i, o, g]
    dperm = np.r_[0:256, 768:1024, 512:768]
    Wd = inputs["Wih_dec"][:, dperm, :] * WSCALE
    wd8 = np.ascontiguousarray(
        Wd.reshape(S, 6, 128, 4, 128).transpose(0, 4, 3, 1, 2)  # [t,p,k,m,q]
    ).astype(f8)
    # fold elu's "-1" into the bias: subtract row sums of the quantized W
    corr = wd8.astype(np.float32).sum(axis=(1, 2))  # [t, m, q]
    bd = ((inputs["bih_dec"] + inputs["bhh_dec"])[:, dperm] * WSCALE).reshape(S, 6, 128) - corr
    bdec = np.ascontiguousarray(
        bd.reshape(S // DCH, DCH, 6, 128).transpose(0, 2, 1, 3)
    ).astype(np.float32)
    wdt = np.ascontiguousarray(
        wd8.reshape(S // DCH, DCH, 128, 4, 6, 128).transpose(0, 2, 1, 3, 4, 5)
    )
    dones = np.ascontiguousarray(
        np.repeat(np.eye(6, dtype=np.float32)[:, :, None], BLOC, axis=2)
    )
    xr = np.ascontiguousarray(
        inputs["x"].reshape(B, 2, 128, S).transpose(2, 3, 1, 0)
    ).astype(f16)
    return wt, benc, eones, wdt, bdec, dones, xr


def kernel(**inputs):
    inputs = {k: np.asarray(v) for k, v in inputs.items()}
    if "nc" not in _STATE:
        _STATE["nc"] = _build_module()
    nc = _STATE["nc"]
    wt, benc, eones, wdt, bdec, dones, xr = _host_prep(inputs)
    in_maps = []
    for c in range(NCORES):
        in_maps.append(
            {
                "wt": wt,
                "wdt": wdt,
                "benc": benc,
                "bdec": bdec,
                "eones": eones,
                "dones": dones,
                "xr": np.ascontiguousarray(xr[:, :, :, c * BLOC : (c + 1) * BLOC]),
            }
        )
    res = bass_utils.run_bass_kernel_spmd(
        nc,
        in_maps,
        core_ids=list(range(NCORES)),
        trace=bool(int(os.environ.get("BASS_KERNEL_TRACE", "0"))),
    )
    _STATE["last_results"] = res
    outs = []
    for c in range(NCORES):
        o = res.results[c]["out"]  # [128, S, 2, BLOC]
        outs.append(
            np.ascontiguousarray(
                o.transpose(3, 2, 0, 1).reshape(BLOC, 2 * 128, S)[:, :, ::-1]
            )
        )
    return np.concatenate(outs, axis=0).astype(np.float32)


# revision 14
# speedup vs baseline: 2.2446x; 1.0980x over previous
"""Trainium2 Bass kernel for nn_ASC_LSTM (per-step LSTM encoder/decoder).

Strategy: data-parallel over batch (32 rows/core x 8 cores). Weights are
replicated, host-scaled by 64 and quantized to fp8 e3m4 (all 16-bit
on-chip surfaces use float16, whose 10-bit mantissa keeps the
recurrent-chain rounding error ~8x below bfloat16), then streamed from
HBM in 2-step chunks (one large DMA per chunk, double-buffered).
Gates are computed transposed ([gate_rows, batch] in PSUM) with all 16
gate chunks of a step accumulated into a single PSUM bank; the per-step
bias is folded in with one K=16 matmul against a one-hot "ones" tensor.
Gate order is [i, f, o, g] so one sigmoid covers chunks 0:12 and one
tanh covers 12:16 (PSUM read + 1/64 descale fused into the activation).
The elu is batched per 8 steps (its Exp needs a different activation
table than sigmoid/tanh); its "-1" is folded into the decoder bias on
the host via row sums of the quantized decoder weights, which is exact
because the skip blend coefficients sum to 1. The sequential skip-blend
chain is re-expressed as a running v-chain (computed as elu blocks
finish) plus 16 independent end corrections u'[4n] = v[n] +
2^-(n+1) * u[60], issued in descending n to match the decoder's
consumption order. Decoder weights/bias prefetch before the encoder
finishes so the DMA stream never idles at the phase boundary.
"""
import os
import sys

import numpy as np
import ml_dtypes

sys.path.insert(0, "/opt/trn_rl_repo")

import concourse.bass as bass
import concourse.tile as tile
from concourse import bacc, mybir
from concourse import bass_utils

B, I, H, S, RES = 256, 256, 512, 64, 4
NCORES = 8
BLOC = B // NCORES  # 32
ECH = 2  # encoder steps per weight-DMA chunk
DCH = 4  # decoder idxs per weight-DMA chunk
WSCALE = 64.0
F16 = mybir.dt.float16
F32 = mybir.dt.float32
FP8 = mybir.dt.float8e3
AF = mybir.ActivationFunctionType

_STATE = {}


def _build_module():
    nc = bacc.Bacc(
        "TRN2",
        target_bir_lowering=False,
        debug=False,
        enable_asserts=False,
        num_devices=NCORES,
    )
    wt_d = nc.dram_tensor("wt", [128, S, 6, 16, 128], FP8, kind="ExternalInput").ap()
    wdt_d = nc.dram_tensor("wdt", [128, S, 4, 6, 128], FP8, kind="ExternalInput").ap()
    x_d = nc.dram_tensor("xr", [128, S, 2, BLOC], F16, kind="ExternalInput").ap()
    benc_d = nc.dram_tensor("benc", [16, S, 128], F16, kind="ExternalInput").ap()
    bdec_d = nc.dram_tensor("bdec", [12, S // 2, 128], F16, kind="ExternalInput").ap()
    eones_d = nc.dram_tensor("eones", [16, 16, BLOC], F16, kind="ExternalInput").ap()
    dones_d = nc.dram_tensor("dones", [12, 6, 2, BLOC], F16, kind="ExternalInput").ap()
    out_d = nc.dram_tensor("out", [128, S, 2, BLOC], F16, kind="ExternalOutput").ap()

    inv = 1.0 / WSCALE

    with tile.TileContext(nc) as tc:
        with (
            tc.tile_pool(name="wenc", bufs=3) as wpool,
            tc.tile_pool(name="wdec", bufs=4) as wdpool,
            tc.tile_pool(name="big", bufs=1) as bigpool,
            tc.tile_pool(name="gates", bufs=2) as gpool,
            tc.tile_pool(name="small", bufs=2) as spool,
            tc.tile_pool(name="psum", bufs=4, space="PSUM") as psum,
        ):
            x_sb = bigpool.tile([128, S, 2, BLOC], F16, tag="xsb")
            nc.sync.dma_start(out=x_sb, in_=x_d)
            benc_sb = bigpool.tile([16, S, 128], F16, tag="benc")
            nc.sync.dma_start(out=benc_sb, in_=benc_d)
            eones_sb = bigpool.tile([16, 16, BLOC], F16, tag="eones")
            nc.sync.dma_start(out=eones_sb, in_=eones_d)
            dones_sb = bigpool.tile([12, 6, 2, BLOC], F16, tag="dones")
            nc.sync.dma_start(out=dones_sb, in_=dones_d)
            bdec_sb = bigpool.tile([12, S // 2, 128], F16, tag="bdec")
            nc.sync.dma_start(out=bdec_sb, in_=bdec_d)

            # f16 h history; becomes u = elu(h)+1 in place, then blended.
            hist = bigpool.tile([128, S, 4, BLOC], F16, tag="hist")
            vtile = bigpool.tile([128, S // RES, 4, BLOC], F16, tag="vt")
            out_sb = bigpool.tile([128, S, 2, BLOC], F16, tag="outsb")

            # prefetch the first decoder weight chunks so the DMA stream
            # has queued work at the encoder->decoder boundary
            dec_w = {}
            for c in range(4):
                wd_sb = wdpool.tile([128, DCH, 4, 6, 128], FP8, tag="wd")
                nc.sync.dma_start(out=wd_sb, in_=wdt_d[:, c * DCH : (c + 1) * DCH])
                dec_w[c] = wd_sb

            # ---------------- encoder scan ----------------
            h_f32 = None
            hb_prev = None
            for c in range(S // ECH):
                w_sb = wpool.tile([128, ECH, 6, 16, 128], FP8, tag="w")
                nc.sync.dma_start(out=w_sb, in_=wt_d[:, c * ECH : (c + 1) * ECH])
                for i in range(ECH):
                    t = c * ECH + i
                    nks = 2 if t == 0 else 6
                    ps = psum.tile([128, 16, BLOC], F32, tag="ps")
                    nc.tensor.matmul(
                        ps, lhsT=benc_sb[:, t], rhs=eones_sb,
                        start=True, stop=False, skip_group_check=True,
                    )
                    # x-dependent matmuls first: the PE queue is in-order, so
                    # issuing these before the h-matmuls lets the PE work
                    # while the previous step's h is still being produced
                    for m in range(16):
                        for k in range(2):
                            nc.tensor.matmul(
                                ps[:, m], lhsT=w_sb[:, i, k, m], rhs=x_sb[:, t, k],
                                start=False, stop=(t == 0 and k == 1),
                                skip_group_check=True,
                            )
                    if t > 0:
                        # g-gate chunks (12:16) first so the tanh activation
                        # overlaps the remaining h-matmuls
                        for m in (12, 13, 14, 15, 0, 1, 2, 3, 4, 5, 6, 7, 8, 9, 10, 11):
                            for k in range(2, 6):
                                nc.tensor.matmul(
                                    ps[:, m], lhsT=w_sb[:, i, k, m], rhs=hb_prev[:, k - 2],
                                    start=False, stop=(k == 5),
                                    skip_group_check=True,
                                )
                    gs = gpool.tile([128, 16, BLOC], F32, tag="gs")
                    nc.scalar.activation(out=gs[:, 12:16], in_=ps[:, 12:16], func=AF.Tanh, scale=inv)
                    nc.scalar.activation(out=gs[:, 0:12], in_=ps[:, 0:12], func=AF.Sigmoid, scale=inv)
                    # c = f*h_prev + i*g ; h = o*tanh(c)
                    cc = spool.tile([128, 4, BLOC], F32, tag="cc")
                    nc.vector.tensor_mul(cc, gs[:, 0:4], gs[:, 12:16])
                    if t > 0:
                        fh = spool.tile([128, 4, BLOC], F32, tag="fh")
                        nc.vector.tensor_mul(fh, gs[:, 4:8], h_f32)
                        nc.vector.tensor_add(cc, cc, fh)
                    tct = spool.tile([128, 4, BLOC], F32, tag="tct")
                    nc.scalar.activation(out=tct, in_=cc, func=AF.Tanh)
                    h_f32 = spool.tile([128, 4, BLOC], F32, tag="h32")
                    nc.vector.tensor_mul(h_f32, tct, gs[:, 8:12])
                    # f16 copies: rolling tile for next-step matmul rhs, and
                    # the history slot (overwritten by elu/blend later)
                    hb_prev = spool.tile([128, 4, BLOC], F16, tag="hb")
                    nc.vector.tensor_copy(out=hb_prev, in_=h_f32)
                    nc.vector.tensor_copy(out=hist[:, t], in_=h_f32)
                    # batched elu for the finished 8-step block:
                    # u = relu(h) + exp(min(h,0))   (the -1 is folded into
                    # the decoder bias on host)
                    if t % 8 == 7:
                        blk = hist[:, t - 7 : t + 1]
                        en = spool.tile([128, 8, 4, BLOC], F16, tag="eneg")
                        nc.vector.tensor_scalar_min(en, blk, 0.0)
                        nc.scalar.activation(out=en, in_=en, func=AF.Exp)
                        nc.vector.tensor_scalar_max(blk, blk, 0.0)
                        nc.vector.tensor_add(blk, blk, en)
                        # v-chain updates for the two blend positions now
                        # available (k = t-7, t-3): v[n] = (u[4n] + v[n-1])/2
                        for k in (t - 7, t - 3):
                            n = k // RES
                            if n == 0:
                                nc.vector.tensor_scalar_mul(vtile[:, 0], hist[:, 0], 0.5)
                            else:
                                nc.vector.tensor_add(vtile[:, n], hist[:, k], vtile[:, n - 1])
                                nc.vector.tensor_scalar_mul(vtile[:, n], vtile[:, n], 0.5)

            # ---------------- skip blend end corrections ----------------
            # u'[4n] = v[n] + 2^-(n+1) * u[60]; descending n matches the
            # decoder's consumption order (idx 4j+3 reads t = 60-4j).
            u60 = spool.tile([128, 4, BLOC], F16, tag="u60")
            nc.vector.tensor_copy(out=u60, in_=hist[:, S - RES])
            for n in range(S // RES - 1, -1, -1):
                bc = spool.tile([128, 4, BLOC], F16, tag="bc")
                nc.vector.tensor_scalar_mul(bc, u60, 0.5 ** (n + 1))
                nc.vector.tensor_add(hist[:, n * RES], vtile[:, n], bc)

            # ---------------- decoder (parallel over idx, 2 idx/batch) ----
            rn_prev = None
            for c in range(S // DCH):
                if c in dec_w:
                    wd_sb = dec_w.pop(c)
                else:
                    wd_sb = wdpool.tile([128, DCH, 4, 6, 128], FP8, tag="wd")
                    nc.sync.dma_start(out=wd_sb, in_=wdt_d[:, c * DCH : (c + 1) * DCH])
                for jp in range(DCH // 2):
                    i0 = c * DCH + jp * 2  # idx pair (i0, i0+1)
                    psd = psum.tile([128, 6, 2, BLOC], F32, tag="psd")
                    nc.tensor.matmul(
                        psd, lhsT=bdec_sb[:, i0 // 2], rhs=dones_sb,
                        start=True, stop=False, skip_group_check=True,
                    )
                    for m in range(6):
                        for j in range(2):
                            tsrc = S - 1 - (i0 + j)
                            for k in range(4):
                                nc.tensor.matmul(
                                    psd[:, m, j],
                                    lhsT=wd_sb[:, jp * 2 + j, k, m],
                                    rhs=hist[:, tsrc, k],
                                    start=False, stop=(k == 3),
                                    skip_group_check=True,
                                )
                    gd = gpool.tile([128, 6, 2, BLOC], F32, tag="gd")
                    nc.scalar.activation(out=gd[:, 0:4], in_=psd[:, 0:4], func=AF.Sigmoid, scale=inv)
                    nc.scalar.activation(out=gd[:, 4:6], in_=psd[:, 4:6], func=AF.Tanh, scale=inv)
                    cd = spool.tile([128, 2, 2, BLOC], F32, tag="cd")
                    nc.vector.tensor_mul(cd, gd[:, 0:2], gd[:, 4:6])
                    nc.scalar.activation(out=cd, in_=cd, func=AF.Tanh)
                    nc.vector.tensor_mul(cd, cd, gd[:, 2:4])  # hd, [128, hh, j, b]
                    hdT = cd.transpose([0, 2, 1, 3])  # [128, j, hh, b] view
                    rn = spool.tile([128, 2, 2, BLOC], F32, tag="rn")
                    if i0 % RES == 0:
                        nc.vector.tensor_copy(out=rn[:, 0], in_=hdT[:, 0])
                    else:
                        nc.vector.tensor_add(rn[:, 0], rn_prev[:, 1], hdT[:, 0])
                    nc.vector.tensor_add(rn[:, 1], rn[:, 0], hdT[:, 1])
                    rn_prev = rn
                    nc.scalar.activation(out=out_sb[:, i0 : i0 + 2], in_=rn, func=AF.Tanh)
                if c % 4 == 3:
                    s0 = (c - 3) * DCH
                    nc.sync.dma_start(
                        out=out_d[:, s0 : s0 + 16], in_=out_sb[:, s0 : s0 + 16]
                    )
    nc.finalize()
    return nc


def _host_prep(inputs):
    f16 = np.float16
    f8 = ml_dtypes.float8_e3m4
    # encoder: gate order [i, f, o, g]
    eperm = np.r_[0:512, 512:1024, 1536:2048, 1024:1536]
    W_all = np.concatenate([inputs["Wih_enc"], inputs["Whh_enc"]], axis=2)[:, eperm, :] * WSCALE
    # [t, 16m, 128q, 6k, 128p] -> [p, t, k, m, q]
    wt = np.ascontiguousarray(
        W_all.reshape(S, 16, 128, 6, 128).transpose(4, 0, 3, 1, 2)
    ).astype(f8)
    benc = np.ascontiguousarray(
        ((inputs["bih_enc"] + inputs["bhh_enc"])[:, eperm] * WSCALE)
        .reshape(S, 16, 128)
        .transpose(1, 0, 2)
    ).astype(f16)
    eones = np.ascontiguousarray(
        np.repeat(np.eye(16, dtype=np.float32)[:, :, None], BLOC, axis=2)
    ).astype(f16)
    # decoder: gate order [i, o, g]
    dperm = np.r_[0:256, 768:1024, 512:768]
    Wd = inputs["Wih_dec"][:, dperm, :] * WSCALE
    wd8 = np.ascontiguousarray(
        Wd.reshape(S, 6, 128, 4, 128).transpose(4, 0, 3, 1, 2)  # [p,t,k,m,q]
    ).astype(f8)
    # fold elu's "-1" into the bias: subtract row sums of the quantized W
    corr = wd8.astype(np.float32).sum(axis=(0, 2))  # [t, m, q]
    bd = ((inputs["bih_dec"] + inputs["bhh_dec"])[:, dperm] * WSCALE).reshape(S, 6, 128) - corr
    # idx-pair packing: bdec[(m*2+j), pair, q] = bd[2*pair+j, m, q]
    bdec = np.ascontiguousarray(
        bd.reshape(S // 2, 2, 6, 128).transpose(2, 1, 0, 3).reshape(12, S // 2, 128)
    ).astype(f16)
    dones = np.ascontiguousarray(
        np.repeat(
            np.eye(12, dtype=np.float32).reshape(12, 6, 2)[:, :, :, None], BLOC, axis=3
        )
    ).astype(f16)
    xr = np.ascontiguousarray(
        inputs["x"].reshape(B, 2, 128, S).transpose(2, 3, 1, 0)
    ).astype(f16)
    return wt, benc, eones, wd8, bdec, dones, xr


def kernel(**inputs):
    inputs = {k: np.asarray(v) for k, v in inputs.items()}
    if "nc" not in _STATE:
        _STATE["nc"] = _build_module()
    nc = _STATE["nc"]
    wt, benc, eones, wdt, bdec, dones, xr = _host_prep(inputs)
    in_maps = []
    for c in range(NCORES):
        in_maps.append(
            {
                "wt": wt,
                "wdt": wdt,
                "benc": benc,
                "bdec": bdec,
                "eones": eones,
                "dones": dones,
                "xr": np.ascontiguousarray(xr[:, :, :, c * BLOC : (c + 1) * BLOC]),
            }
        )
    res = bass_utils.run_bass_kernel_spmd(
        nc,
        in_maps,
        core_ids=list(range(NCORES)),
        trace=bool(int(os.environ.get("BASS_KERNEL_TRACE", "0"))),
    )
    _STATE["last_results"] = res
    outs = []
    for c in range(NCORES):
        o = np.asarray(res.results[c]["out"]).astype(np.float32)  # [128, S, 2, BLOC]
        outs.append(
            np.ascontiguousarray(
                o.transpose(3, 2, 0, 1).reshape(BLOC, 2 * 128, S)[:, :, ::-1]
            )
        )
    return np.concatenate(outs, axis=0).astype(np.float32)
